# revision 9
# baseline (speedup 1.0000x reference)
"""Trainium2 Bass kernel for the pairwise-similarity exp-sum loss (v3).

reference math (BETA=10, x: [16384, 512] f32):
    norms_i  = sum_k x[i,k]^2
    pair[i,j] = 2*x_i.x_j + norms_i + norms_j
    lhs = (1/BETA^256) * sum_ij exp(pair/40) / N
    rhs = (2/(BETA-.5)^256) * sum_i exp(norms_i/38)
    out = lhs - rhs
(The two scale coefficients underflow to 0.0 in float32, matching the
reference's own f32 arithmetic; the kernel still computes both big sums
honestly on hardware.)

v3 = v2 with the edges tightened (steady-state tile math unchanged):
  * w0 (own-block triangle) tiles run in REVERSED order (t=15..0): the
    first tiles need only the tail of the moving operand and one wts
    group, so compute starts ~7us earlier.  mts is loaded as hi/lo
    halves per contraction chunk to match.
  * The n/40 AllGather chain rides the SYNC queue (its barrier-flag
    DMAs live there anyway), so the gpsimd queue never blocks and
    carries all wts group loads instead of the Scalar queue (v2 kept
    17 DMA issues + waits on the ACT-critical queue).
  * The AllGather-dependent bias/correction tables are emitted mid
    stream (after sched tile 32, ~40us after the collective lands)
    instead of at the end: the post-loop epilogue shrinks to the two
    cacc multiplies + final reduction.
  * Same tile pipeline as v2: fp8 DoubleRow mains into [128,2048]
    PSUM, ACT Exp (per-partition j-norm bias), DVE STT weighted
    reduce; every PE_NTH'th w123 tile takes the PE-bias + ACT-accum
    path to keep DVE under the PE/ACT roofline.
Each core emits 128 lhs + 128 rhs partial lanes; the host sums lanes
and cores and applies the final affine combine (in f32, where both
coefficients underflow to exactly 0 like the reference).
"""

import os
import sys

sys.path.insert(0, "/opt/trn_rl_repo")

import numpy as np
import ml_dtypes

import concourse.bass as bass
import concourse.bacc as bacc
import concourse.mybir as mybir
import concourse.tile as tile
from concourse.bass_utils import run_bass_kernel_spmd

dt = mybir.dt
AF = mybir.ActivationFunctionType
ALU = mybir.AluOpType

N = 16384
D = 512
NCORES = 8
ROWS = N // NCORES
BETA = 10.0

# every PE_NTH w123 tile takes the PE-bias + ACT-accum path (no DVE),
# balancing the DVE scalar_tensor_tensor against PE
PE_NTH = int(os.environ.get("V3PENTH", "12"))
# sched index after which the AllGather-dependent tables are emitted
CORR_AT = int(os.environ.get("V3CORRAT", "32"))


def build_program(n=N):
    rows = n // NCORES          # own rows per core (2048)
    W = 2048                    # PSUM processing tile width (4 banks)
    nrt = rows // 128           # own row-tiles (16)
    kc = D // 128               # 4 contraction chunks (2 DoubleRow matmuls)
    half = NCORES // 2
    npan = half + 1             # staged panels w=0..4
    jt_n = npan * nrt           # 80 staged j-tiles
    wcols = npan * rows         # staged wT columns
    ln2 = float(np.log(2.0))
    red_dt = dt.bfloat16

    nc = bacc.Bacc(
        "TRN2",
        target_bir_lowering=False,
        debug=False,
        enable_asserts=False,
        num_devices=NCORES,
    )

    wT = nc.dram_tensor("wT", [D, wcols], dt.float8e4, kind="ExternalInput")
    xo = nc.dram_tensor("xo", [rows, D], dt.bfloat16, kind="ExternalInput")
    # per-core w4 bias-column offset: 8 for cores >= 4, else 0
    sw4 = nc.dram_tensor("sw4", [1, 1], dt.uint32, kind="ExternalInput")
    ident = nc.dram_tensor("ident", [128, 128], dt.bfloat16, kind="ExternalInput")
    po = nc.dram_tensor("po", [256], dt.float32, kind="ExternalOutput")

    wT_ap = wT.ap()
    po_lhs = po.ap()[0:128].rearrange("(p o) -> p o", o=1)
    po_rhs = po.ap()[128:256].rearrange("(p o) -> p o", o=1)

    # ps-tile schedule: w0 triangle tiles REVERSED, w123 full, w4 pairs
    sched = (
        [("w0", t) for t in range(nrt - 1, -1, -1)]
        + [("w123", jt) for jt in range(nrt, 4 * nrt)]
        + [("w4", s) for s in range(nrt // 2)]
    )
    # stationary group needed at sched position i//8 (w0 reversed uses
    # group 1 then group 0; w123 groups 2..7; w4 group 8)
    gseq = [1, 0, 2, 3, 4, 5, 6, 7, 8]
    n_acc = len(sched) + nrt // 2

    with tile.TileContext(nc) as tc:
        with (
            tc.tile_pool(name="dram", bufs=1, space="DRAM") as dram,
            tc.tile_pool(name="const", bufs=1) as const,
            tc.tile_pool(name="stat", bufs=1) as stat,
            tc.tile_pool(name="xop", bufs=3) as xop,
            tc.tile_pool(name="sqp", bufs=2) as sqp,
            tc.tile_pool(name="wtp", bufs=3) as wtp,
            tc.tile_pool(name="mtp", bufs=1) as mtp,
            tc.tile_pool(name="etp", bufs=18) as etp,
            tc.tile_pool(name="ttp", bufs=2) as ttp,
            tc.tile_pool(name="accp", bufs=1) as accp,
            tc.tile_pool(name="mainps", bufs=2, space="PSUM") as mainps,
        ):
            # ---------------- prelude: operand staging ----------------
            # Queue layout: the AG barrier-flag DMAs ride the sync queue,
            # and ANY bulk DMA queued ahead of them stretches the AG
            # rendezvous to 60us+ (measured) — so sync carries ONLY the
            # xo loads + n40_own ahead of the flags.  The moving-operand
            # halves ride the Scalar queue (issue-cost only, ~0.7us each);
            # stationary groups + the collective chain ride gpsimd.
            mts_half = {}  # (kp, h) -> tile covering m in [1024h, 1024h+1024)
            for kp in range(kc // 2):
                mtk = mtp.tile([128, 2, 1024], dt.float8e4, tag=f"mth{kp}")
                nc.scalar.dma_start(
                    out=mtk[:],
                    in_=wT_ap[kp * 256 : (kp + 1) * 256, 1024:2048].rearrange(
                        "(g p) c -> p g c", g=2
                    ),
                )
                mts_half[(kp, 1)] = mtk
            identT = const.tile([128, 128], red_dt)
            nc.scalar.dma_start(out=identT[:], in_=ident.ap())

            def load_wts_group(gc0, gcw, eng=None):
                eng = eng or nc.gpsimd
                wts = []
                for kp in range(kc // 2):
                    wtk = wtp.tile([128, 2, gcw], dt.float8e4, tag=f"wt{kp}")
                    eng.dma_start(
                        out=wtk[:],
                        in_=wT_ap[
                            kp * 256 : (kp + 1) * 256, gc0 : gc0 + gcw
                        ].rearrange("(g p) c -> p g c", g=2),
                    )
                    wts.append(wtk)
                return wts

            # first FOUR stationary groups load ahead of the AllGather on
            # gpsimd (later in-loop prefetches queue behind the AG but
            # complete long before the w123 demand catches up)
            wgroups = [(k * 1024, 1024) for k in range(8)] + [(8192, 2048)]
            wpref = {g: load_wts_group(*wgroups[g]) for g in gseq[:4]}

            # row norms: xo groups (reversed) on sync; Squares split
            # between ACT (last 2 tiles of each group, needed first) and
            # DVE (first 2).
            ns = stat.tile([128, nrt], dt.float32)
            ns40 = stat.tile([128, nrt], dt.float32)
            ns40_2 = stat.tile([128, nrt], dt.float32)
            comb = stat.tile([128, 128], red_dt)
            nc.vector.memset(comb[:], 0.0)
            combT = stat.tile([128, 128], red_dt)
            w_row = const.tile([1, rows], red_dt)
            ln2c = const.tile([128, 1], dt.float32)
            nc.vector.memset(ln2c[:], ln2)
            ones_r = const.tile([1, 128], red_dt)
            nc.vector.memset(ones_r[:], 1.0)
            c128b = const.tile([128, 1], dt.float32)   # 256/(2*BETA)
            nc.vector.memset(c128b[:], 256.0 / (2.0 * BETA))
            cm256 = const.tile([128, 1], dt.float32)
            nc.vector.memset(cm256[:], -256.0)

            xo_g = xo.ap().rearrange("(g t p) d -> g p t d", p=128, t=4)
            first_mts_lo = True
            for gi, g4 in enumerate(range(nrt // 4 - 1, -1, -1)):
                xot = xop.tile([128, 4, D], dt.bfloat16, tag="xot")
                nc.sync.dma_start(out=xot[:], in_=xo_g[g4])
                if first_mts_lo:
                    # low mts halves follow the high ones on the DVE queue
                    for kp in range(kc // 2):
                        mtk = mtp.tile([128, 2, 1024], dt.float8e4,
                                       tag=f"mtl{kp}")
                        nc.scalar.dma_start(
                            out=mtk[:],
                            in_=wT_ap[
                                kp * 256 : (kp + 1) * 256, 0:1024
                            ].rearrange("(g p) c -> p g c", g=2),
                        )
                        mts_half[(kp, 0)] = mtk
                    first_mts_lo = False
                for tt in range(3, -1, -1):
                    t = g4 * 4 + tt
                    if tt >= 2:  # ACT path (needed first in reversed order)
                        nc.scalar.activation(
                            xot[:, tt], xot[:, tt], AF.Square,
                            accum_out=ns[:, t : t + 1],
                        )
                    else:        # DVE path
                        sq = sqp.tile([128, D], dt.float32, tag="sq")
                        nc.vector.tensor_tensor(
                            out=sq[:], in0=xot[:, tt], in1=xot[:, tt],
                            op=ALU.mult,
                        )
                        nc.vector.tensor_reduce(
                            out=ns[:, t : t + 1], in_=sq[:],
                            op=ALU.add, axis=mybir.AxisListType.X,
                        )
                g0, g1 = g4 * 4, g4 * 4 + 4
                nc.scalar.activation(
                    ns40[:, g0:g1], ns[:, g0:g1], AF.Copy,
                    scale=1.0 / (4.0 * BETA),
                )
                nc.scalar.activation(
                    ns40_2[:, g0:g1], ns40[:, g0:g1], AF.Identity,
                    bias=ln2c[:],
                )
                nc.scalar.activation(
                    comb[:, g0:g1], ns[:, g0:g1], AF.Exp,
                    scale=1.0 / (4.0 * BETA),
                )

            # n/40 AllGather chain (gpsimd: collectives must ride gpsimd
            # for NRT's straight-line collective ordering)
            n40_own = dram.tile([rows], dt.float32)
            nc.sync.dma_start(
                out=n40_own[:].rearrange("(p t) -> p t", p=128), in_=ns40[:]
            )
            n40_full = dram.tile([n], dt.float32, addr_space="Shared")
            nc.gpsimd.collective_compute(
                "AllGather",
                ALU.bypass,
                replica_groups=[list(range(NCORES))],
                ins=[n40_own[:].opt()],
                outs=[n40_full[:].opt()],
            )
            n40_dbl = dram.tile([2 * n], dt.float32)
            nc.gpsimd.dma_start(out=n40_dbl[0:n], in_=n40_full[:])
            nc.gpsimd.dma_start(out=n40_dbl[n : 2 * n], in_=n40_full[:])
            pid = nc.gpsimd.partition_id()
            coff = pid * rows
            n40_rot = const.tile([128, jt_n], dt.float32)
            nc.gpsimd.dma_start(
                out=n40_rot[:].rearrange("q (c t) -> q c t", t=nrt),
                in_=n40_dbl[bass.ds(coff, npan * rows)].rearrange(
                    "(c p t) -> p c t", p=128, t=nrt
                ),
            )

            # rhs partial: sum exp(norms/38) over own rows
            rs = stat.tile([128, 1], dt.float32)
            trash_n = stat.tile([128, nrt], dt.float32)
            nc.scalar.activation(
                trash_n[:], ns[:], AF.Exp, scale=1.0 / (4.0 * BETA - 2.0),
                accum_out=rs[:],
            )

            # PE-bias moving rows: bf16 two-term split of r = n/2 - 256
            rf = stat.tile([128, nrt], dt.float32)
            nc.scalar.activation(
                rf[:], ns[:], AF.Identity, scale=0.5, bias=cm256[:]
            )
            nc.scalar.activation(comb[:, 16:32], rf[:], AF.Copy)
            r1f = stat.tile([128, nrt], dt.float32)
            nc.scalar.activation(r1f[:], comb[:, 16:32], AF.Copy)
            r2f = stat.tile([128, nrt], dt.float32)
            nc.vector.tensor_tensor(
                out=r2f[:], in0=rf[:], in1=r1f[:], op=ALU.subtract
            )
            nc.scalar.activation(comb[:, 32:48], r2f[:], AF.Copy)

            r12 = const.tile([2, rows], red_dt)
            ones2 = const.tile([2, 128], red_dt)
            nc.vector.memset(ones2[:], 1.0)

            # ---------------- main loop ----------------
            def chunks(m0, m1):
                out = []
                while m0 < m1:
                    m2 = min((m0 // 512 + 1) * 512, m1)
                    out.append((m0, m2))
                    m0 = m2
                return out

            acc = accp.tile([128, n_acc], dt.float32)
            w_bc = const.tile([128, rows], red_dt)
            n40_rot2 = const.tile([128, jt_n], dt.float32)
            n40_rot2c = const.tile([128, jt_n], dt.float32)
            n40_w4 = const.tile([128, nrt], dt.float32)
            corr48 = stat.tile([128, 48], dt.float32)
            corr16 = stat.tile([128, nrt], dt.float32)
            deferred = []

            def emit_tail(col, et, mlo, mhi=W):
                tt_o = ttp.tile([128, W], red_dt, tag="tt")
                nc.vector.scalar_tensor_tensor(
                    out=tt_o[:, mlo:mhi], in0=et[:, mlo:mhi], scalar=1.0,
                    in1=w_bc[:, mlo:mhi], op0=ALU.mult, op1=ALU.mult,
                    accum_out=acc[:, col : col + 1],
                )

            def emit_corr_tables():
                # AllGather-dependent bias tables; emitted mid-stream,
                # long after the collective has completed
                nc.scalar.activation(
                    n40_rot2[:], n40_rot[:], AF.Identity, bias=ln2c[:]
                )
                nc.scalar.activation(
                    n40_rot2c[:], n40_rot2[:], AF.Identity, bias=c128b[:]
                )
                n40_w4x = const.tile([128, 24], dt.float32)
                nc.sync.dma_start(out=n40_w4x[:, 0:16], in_=n40_rot2[:, 64:80])
                nc.sync.dma_start(out=n40_w4x[:, 16:24], in_=n40_rot2[:, 64:72])
                tmp = nc.gpsimd.alloc_register("sw4reg")
                nc.gpsimd.reg_load(tmp, sw4.ap()[0:1, 0:1])
                troff = nc.gpsimd.snap(tmp, donate=True, min_val=0, max_val=8)
                nc.gpsimd.dma_start(
                    out=n40_w4[:], in_=n40_w4x[:, bass.ds(troff, 16)]
                )
                argt = stat.tile([128, 48], dt.float32)
                nc.scalar.activation(argt[:], n40_rot2[:, 16:64], AF.Copy)
                for jt in range(nrt, 4 * nrt):
                    if PE_NTH > 0 and (jt - nrt) % PE_NTH == 2:
                        nc.scalar.activation(
                            argt[:, jt - nrt : jt - nrt + 1],
                            n40_rot2c[:, jt : jt + 1],
                            AF.Copy,
                        )
                nc.scalar.activation(corr48[:], argt[:], AF.Exp)
                nc.scalar.activation(corr16[:], n40_w4[:], AF.Exp)

            wts = None
            for i, (kind, idx) in enumerate(sched):
                if i % 8 == 0 and i // 8 < len(gseq):
                    g = gseq[i // 8]
                    wts = wpref.pop(g)
                    if i // 8 + 1 < len(gseq):
                        nxt = gseq[i // 8 + 1]
                        if nxt not in wpref:
                            wpref[nxt] = load_wts_group(*wgroups[nxt])

                if kind == "w0":
                    t = idx
                    mlo = 128 * t
                    mm = [(m0, m1, (t % 8) * 128) for m0, m1 in chunks(mlo, W)]
                    acts = [(mlo, mlo + 128, ns40[:, t : t + 1])]
                    if t < nrt - 1:
                        acts.append((mlo + 128, W, ns40_2[:, t : t + 1]))
                elif kind == "w123":
                    jt = idx
                    mlo = 0
                    jcol = ((jt - nrt) % 8) * 128
                    mm = [(m0, m1, jcol) for m0, m1 in chunks(0, W)]
                    acts = [(0, W, 0.0)]
                else:
                    s = idx
                    mlo = 0
                    mm = [(m0, m1, s * 128) for m0, m1 in chunks(0, 1024)] + [
                        (m0, m1, (s + 8) * 128) for m0, m1 in chunks(1024, W)
                    ]
                    acts = [(0, 1024, 0.0), (1024, W, 0.0)]

                is_pe = (
                    PE_NTH > 0
                    and kind == "w123"
                    and (idx - nrt) % PE_NTH == 2
                )
                ps = mainps.tile([128, W], dt.float32, tag="ps")
                for m0, m1, jcol in mm:
                    h = m0 // 1024
                    for kp in range(kc // 2):
                        nc.tensor.matmul(
                            ps[:, m0:m1],
                            wts[kp][:, :, jcol : jcol + 128],
                            mts_half[(kp, h)][:, :, m0 - 1024 * h : m1 - 1024 * h],
                            start=(kp == 0),
                            stop=(kp == kc // 2 - 1) and not is_pe,
                            perf_mode=mybir.MatmulPerfMode.DoubleRow,
                        )
                    if is_pe:
                        nc.tensor.matmul(
                            ps[:, m0:m1],
                            ones2[:],
                            r12[:, m0:m1],
                            start=False,
                            stop=True,
                        )
                et = etp.tile([128, W], red_dt, tag="et")
                if is_pe:
                    nc.scalar.activation(
                        et[:],
                        ps[:],
                        AF.Exp,
                        scale=1.0 / (2.0 * BETA),
                        accum_out=acc[:, i : i + 1],
                    )
                    if i == CORR_AT:
                        emit_corr_tables()
                    continue
                for m0, m1, bias_ap in acts:
                    nc.scalar.activation(
                        et[:, m0:m1],
                        ps[:, m0:m1],
                        AF.Exp,
                        bias=bias_ap,
                        scale=1.0 / (2.0 * BETA),
                    )

                if i < nrt:
                    # defer w0 STT tails until w_bc exists
                    deferred.append((i, et, mlo))
                    if i == nrt - 1:
                        tps = mainps.tile([128, 2048], red_dt, tag="ps")
                        nc.tensor.transpose(
                            tps[:, 0:128], comb[:], identT[:]
                        )
                        nc.scalar.activation(
                            combT[:], tps[:, 0:128], AF.Copy
                        )
                        nc.scalar.dma_start(
                            out=w_row[0:1, :], in_=combT[0:16, :]
                        )
                        nc.scalar.dma_start(
                            out=r12[0:1, :], in_=combT[16:32, :]
                        )
                        nc.scalar.dma_start(
                            out=r12[1:2, :], in_=combT[32:48, :]
                        )
                        wps = mainps.tile([128, W], dt.float32, tag="ps")
                        for b in range(W // 512):
                            nc.tensor.matmul(
                                wps[:, b * 512 : (b + 1) * 512],
                                ones_r[:],
                                w_row[0:1, b * 512 : (b + 1) * 512],
                                start=True,
                                stop=True,
                            )
                        nc.scalar.activation(w_bc[:], wps[:], AF.Copy)
                        for d_i, d_et, d_mlo in deferred:
                            emit_tail(d_i, d_et, d_mlo)
                elif kind == "w4":
                    emit_tail(64 + idx, et, 0, 1024)
                    emit_tail(72 + idx, et, 1024, W)
                else:
                    emit_tail(i, et, mlo)
                if i == CORR_AT:
                    emit_corr_tables()

            # ---------------- final column correction + reduction -------
            cacc = stat.tile([128, 64], dt.float32)
            nc.vector.tensor_tensor(
                out=cacc[:, 0:48], in0=acc[:, 16:64], in1=corr48[:],
                op=ALU.mult,
            )
            nc.vector.tensor_tensor(
                out=cacc[:, 48:64], in0=acc[:, 64:80], in1=corr16[:],
                op=ALU.mult,
            )
            af0 = stat.tile([128, 1], dt.float32)
            nc.vector.tensor_reduce(
                out=af0[:], in_=acc[:, 0:16], op=ALU.add,
                axis=mybir.AxisListType.X,
            )
            af1 = stat.tile([128, 1], dt.float32)
            nc.vector.tensor_reduce(
                out=af1[:], in_=cacc[:], op=ALU.add, axis=mybir.AxisListType.X
            )
            af = stat.tile([128, 1], dt.float32)
            nc.vector.tensor_tensor(
                out=af[:], in0=af0[:], in1=af1[:], op=ALU.add
            )
            nc.sync.dma_start(out=po_lhs, in_=af[:])
            nc.sync.dma_start(out=po_rhs, in_=rs[:])

    nc.compile()
    return nc


_NC_CACHE = None


def _get_nc():
    global _NC_CACHE
    if _NC_CACHE is None:
        _NC_CACHE = build_program()
    return _NC_CACHE


def _run(x: np.ndarray, **spmd_kwargs):
    assert x.shape == (N, D)
    x = np.asarray(x, dtype=np.float32)
    xT = np.ascontiguousarray(x.T)
    wT_bf = xT.astype(ml_dtypes.float8_e4m3)

    in_maps = []
    for c in range(NCORES):
        sl = slice(c * ROWS, (c + 1) * ROWS)
        stg = np.roll(wT_bf, -c * ROWS, axis=1)[:, : (NCORES // 2 + 1) * ROWS]
        if c >= NCORES // 2:
            # swap the w4 panel halves so cores c and c+4 jointly cover all
            # four quadrants of their shared block pair
            w4 = stg[:, 4 * ROWS :].copy()
            stg = np.concatenate(
                [stg[:, : 4 * ROWS], w4[:, ROWS // 2 :], w4[:, : ROWS // 2]],
                axis=1,
            )
        in_maps.append(
            {
                "wT": np.ascontiguousarray(stg),
                "xo": np.ascontiguousarray(
                    x[sl].astype(ml_dtypes.bfloat16)
                ),
                "sw4": np.array(
                    [[8 if c >= NCORES // 2 else 0]], dtype=np.uint32
                ),
                "ident": np.eye(128, dtype=ml_dtypes.bfloat16),
            }
        )

    nc = _get_nc()
    res = run_bass_kernel_spmd(nc, in_maps, core_ids=list(range(NCORES)), **spmd_kwargs)

    lhs_tot = np.float32(0.0)
    rhs_tot = np.float32(0.0)
    for c in range(NCORES):
        lanes = np.asarray(res.results[c]["po"], dtype=np.float32).reshape(-1)
        lhs_tot = np.float32(lhs_tot + lanes[0:128].sum(dtype=np.float32))
        rhs_tot = np.float32(rhs_tot + lanes[128:256].sum(dtype=np.float32))

    # mirror the reference's f32 arithmetic (both coefficients underflow to 0)
    with np.errstate(under="ignore"):
        coef_l = np.float32(1.0 / BETA ** (D / 2))
        coef_r = np.float32(2.0 / (BETA - 0.5) ** (D / 2))
    out = np.float32(coef_l * lhs_tot / np.float32(N) - coef_r * rhs_tot)
    return out, res, (lhs_tot, rhs_tot)


def kernel(x: np.ndarray) -> np.ndarray:
    out, _, _ = _run(x)
    return out


def kernel_traced(x: np.ndarray, trace_cores=None):
    out, res, sums = _run(
        x,
        trace=True,
        trace_cores=trace_cores if trace_cores is not None else [0],
    )
    return out, res, sums


# revision 10
# speedup vs baseline: 1.2186x; 1.2186x over previous
"""Trainium2 Bass kernel for the pairwise-similarity exp-sum loss (v3).

reference math (BETA=10, x: [16384, 512] f32):
    norms_i  = sum_k x[i,k]^2
    pair[i,j] = 2*x_i.x_j + norms_i + norms_j
    lhs = (1/BETA^256) * sum_ij exp(pair/40) / N
    rhs = (2/(BETA-.5)^256) * sum_i exp(norms_i/38)
    out = lhs - rhs
(The two scale coefficients underflow to 0.0 in float32, matching the
reference's own f32 arithmetic; the kernel still computes both big sums
honestly on hardware.)

v3 = v2 with the edges tightened (steady-state tile math unchanged):
  * w0 (own-block triangle) tiles run in REVERSED order (t=15..0): the
    first tiles need only the tail of the moving operand and one wts
    group, so compute starts ~7us earlier.  mts is loaded as hi/lo
    halves per contraction chunk to match.
  * The n/40 AllGather chain rides the SYNC queue (its barrier-flag
    DMAs live there anyway), so the gpsimd queue never blocks and
    carries all wts group loads instead of the Scalar queue (v2 kept
    17 DMA issues + waits on the ACT-critical queue).
  * The AllGather-dependent bias/correction tables are emitted mid
    stream (after sched tile 32, ~40us after the collective lands)
    instead of at the end: the post-loop epilogue shrinks to the two
    cacc multiplies + final reduction.
  * Same tile pipeline as v2: fp8 DoubleRow mains into [128,2048]
    PSUM, ACT Exp (per-partition j-norm bias), DVE STT weighted
    reduce; every PE_NTH'th w123 tile takes the PE-bias + ACT-accum
    path to keep DVE under the PE/ACT roofline.
Each core emits 128 lhs + 128 rhs partial lanes; the host sums lanes
and cores and applies the final affine combine (in f32, where both
coefficients underflow to exactly 0 like the reference).
"""

import os
import sys

sys.path.insert(0, "/opt/trn_rl_repo")

import numpy as np
import ml_dtypes

import concourse.bass as bass
import concourse.bacc as bacc
import concourse.mybir as mybir
import concourse.tile as tile
from concourse.bass_utils import run_bass_kernel_spmd

dt = mybir.dt
AF = mybir.ActivationFunctionType
ALU = mybir.AluOpType

N = 16384
D = 512
NCORES = 8
ROWS = N // NCORES
BETA = 10.0

# every PE_NTH w123 tile takes the PE-bias + ACT-accum path (no DVE),
# balancing the DVE scalar_tensor_tensor against PE
PE_NTH = int(os.environ.get("V3PENTH", "12"))
# sched index after which the AllGather-dependent tables are emitted
CORR_AT = int(os.environ.get("V3CORRAT", "40"))


def build_program(n=N):
    rows = n // NCORES          # own rows per core (2048)
    W = 2048                    # PSUM processing tile width (4 banks)
    nrt = rows // 128           # own row-tiles (16)
    kc = D // 128               # 4 contraction chunks (2 DoubleRow matmuls)
    half = NCORES // 2
    npan = half + 1             # staged panels w=0..4
    jt_n = npan * nrt           # 80 staged j-tiles
    wcols = npan * rows         # staged wT columns
    ln2 = float(np.log(2.0))
    red_dt = dt.bfloat16

    nc = bacc.Bacc(
        "TRN2",
        target_bir_lowering=False,
        debug=False,
        enable_asserts=False,
        num_devices=NCORES,
    )

    wT = nc.dram_tensor("wT", [D, wcols], dt.float8e4, kind="ExternalInput")
    xo = nc.dram_tensor("xo", [rows, D], dt.bfloat16, kind="ExternalInput")
    # per-core w4 bias-column offset: 8 for cores >= 4, else 0
    sw4 = nc.dram_tensor("sw4", [1, 1], dt.uint32, kind="ExternalInput")
    ident = nc.dram_tensor("ident", [128, 128], dt.bfloat16, kind="ExternalInput")
    po = nc.dram_tensor("po", [256], dt.float32, kind="ExternalOutput")

    wT_ap = wT.ap()
    po_lhs = po.ap()[0:128].rearrange("(p o) -> p o", o=1)
    po_rhs = po.ap()[128:256].rearrange("(p o) -> p o", o=1)

    # ps-tile schedule: w0 triangle tiles REVERSED, w123 full, w4 pairs
    sched = (
        [("w0", t) for t in range(nrt - 1, -1, -1)]
        + [("w123", jt) for jt in range(nrt, 4 * nrt)]
        + [("w4", s) for s in range(nrt // 2)]
    )
    # stationary group needed at sched position i//8 (w0 reversed uses
    # group 1 then group 0; w123 groups 2..7; w4 group 8)
    gseq = [1, 0, 2, 3, 4, 5, 6, 7, 8]
    n_acc = len(sched) + nrt // 2

    with tile.TileContext(nc) as tc:
        with (
            tc.tile_pool(name="dram", bufs=1, space="DRAM") as dram,
            tc.tile_pool(name="const", bufs=1) as const,
            tc.tile_pool(name="stat", bufs=1) as stat,
            tc.tile_pool(name="xop", bufs=3) as xop,
            tc.tile_pool(name="sqp", bufs=2) as sqp,
            tc.tile_pool(name="wtp", bufs=6) as wtp,
            tc.tile_pool(name="mtp", bufs=1) as mtp,
            tc.tile_pool(name="etp", bufs=18) as etp,
            tc.tile_pool(name="ttp", bufs=2) as ttp,
            tc.tile_pool(name="accp", bufs=1) as accp,
            tc.tile_pool(name="mainps", bufs=2, space="PSUM") as mainps,
        ):
            # ---------------- prelude: operand staging ----------------
            # Queue layout: the AG barrier-flag DMAs ride the sync queue,
            # and ANY bulk DMA queued ahead of them stretches the AG
            # rendezvous to 60us+ (measured) — so sync carries ONLY the
            # xo loads + n40_own ahead of the flags.  The moving-operand
            # halves ride the Scalar queue (issue-cost only, ~0.7us each);
            # stationary groups + the collective chain ride gpsimd.
            mts_half = {}  # (kp, h) -> tile covering m in [1024h, 1024h+1024)
            for kp in range(kc // 2):
                mtk = mtp.tile([128, 2, 1024], dt.float8e4, tag=f"mth{kp}")
                nc.scalar.dma_start(
                    out=mtk[:],
                    in_=wT_ap[kp * 256 : (kp + 1) * 256, 1024:2048].rearrange(
                        "(g p) c -> p g c", g=2
                    ),
                )
                mts_half[(kp, 1)] = mtk
            identT = const.tile([128, 128], red_dt)
            nc.scalar.dma_start(out=identT[:], in_=ident.ap())

            def load_wts_group(gc0, gcw, eng=None):
                eng = eng or nc.gpsimd
                wts = []
                for kp in range(kc // 2):
                    wtk = wtp.tile([128, 2, gcw], dt.float8e4, tag=f"wt{kp}")
                    eng.dma_start(
                        out=wtk[:],
                        in_=wT_ap[
                            kp * 256 : (kp + 1) * 256, gc0 : gc0 + gcw
                        ].rearrange("(g p) c -> p g c", g=2),
                    )
                    wts.append(wtk)
                return wts

            # first FIVE stationary groups load ahead of the AllGather on
            # gpsimd; the pool holds 6 buffers per chunk so none of these
            # dma_starts blocks the queue, and in-loop prefetches stay 4
            # groups ahead of demand
            wgroups = [(k * 1024, 1024) for k in range(8)] + [(8192, 2048)]
            wpref = {g: load_wts_group(*wgroups[g]) for g in gseq[:5]}

            # row norms: xo groups (reversed) on sync; Squares split
            # between ACT (last 2 tiles of each group, needed first) and
            # DVE (first 2).
            ns = stat.tile([128, nrt], dt.float32)
            ns40 = stat.tile([128, nrt], dt.float32)
            ns40_2 = stat.tile([128, nrt], dt.float32)
            comb = stat.tile([128, 128], red_dt)
            nc.vector.memset(comb[:], 0.0)
            combT = stat.tile([128, 128], red_dt)
            w_row = const.tile([1, rows], red_dt)
            ln2c = const.tile([128, 1], dt.float32)
            nc.vector.memset(ln2c[:], ln2)
            ones_r = const.tile([1, 128], red_dt)
            nc.vector.memset(ones_r[:], 1.0)
            c128b = const.tile([128, 1], dt.float32)   # 256/(2*BETA)
            nc.vector.memset(c128b[:], 256.0 / (2.0 * BETA))
            cm256 = const.tile([128, 1], dt.float32)
            nc.vector.memset(cm256[:], -256.0)

            xo_g = xo.ap().rearrange("(g t p) d -> g p t d", p=128, t=4)
            first_mts_lo = True
            for gi, g4 in enumerate(range(nrt // 4 - 1, -1, -1)):
                xot = xop.tile([128, 4, D], dt.bfloat16, tag="xot")
                nc.sync.dma_start(out=xot[:], in_=xo_g[g4])
                if first_mts_lo:
                    # low mts halves follow the high ones on the DVE queue
                    for kp in range(kc // 2):
                        mtk = mtp.tile([128, 2, 1024], dt.float8e4,
                                       tag=f"mtl{kp}")
                        nc.scalar.dma_start(
                            out=mtk[:],
                            in_=wT_ap[
                                kp * 256 : (kp + 1) * 256, 0:1024
                            ].rearrange("(g p) c -> p g c", g=2),
                        )
                        mts_half[(kp, 0)] = mtk
                    first_mts_lo = False
                for tt in range(3, -1, -1):
                    t = g4 * 4 + tt
                    if tt >= 2:  # ACT path (needed first in reversed order)
                        nc.scalar.activation(
                            xot[:, tt], xot[:, tt], AF.Square,
                            accum_out=ns[:, t : t + 1],
                        )
                    else:        # DVE path
                        sq = sqp.tile([128, D], dt.float32, tag="sq")
                        nc.vector.tensor_tensor(
                            out=sq[:], in0=xot[:, tt], in1=xot[:, tt],
                            op=ALU.mult,
                        )
                        nc.vector.tensor_reduce(
                            out=ns[:, t : t + 1], in_=sq[:],
                            op=ALU.add, axis=mybir.AxisListType.X,
                        )
                g0, g1 = g4 * 4, g4 * 4 + 4
                nc.scalar.activation(
                    ns40[:, g0:g1], ns[:, g0:g1], AF.Copy,
                    scale=1.0 / (4.0 * BETA),
                )
                nc.scalar.activation(
                    ns40_2[:, g0:g1], ns40[:, g0:g1], AF.Identity,
                    bias=ln2c[:],
                )
                nc.scalar.activation(
                    comb[:, g0:g1], ns[:, g0:g1], AF.Exp,
                    scale=1.0 / (4.0 * BETA),
                )

            # n/40 AllGather chain (gpsimd: collectives must ride gpsimd
            # for NRT's straight-line collective ordering)
            n40_own = dram.tile([rows], dt.float32)
            nc.sync.dma_start(
                out=n40_own[:].rearrange("(p t) -> p t", p=128), in_=ns40[:]
            )
            n40_full = dram.tile([n], dt.float32, addr_space="Shared")
            nc.gpsimd.collective_compute(
                "AllGather",
                ALU.bypass,
                replica_groups=[list(range(NCORES))],
                ins=[n40_own[:].opt()],
                outs=[n40_full[:].opt()],
            )
            n40_dbl = dram.tile([2 * n], dt.float32)
            nc.gpsimd.dma_start(out=n40_dbl[0:n], in_=n40_full[:])
            nc.gpsimd.dma_start(out=n40_dbl[n : 2 * n], in_=n40_full[:])
            pid = nc.gpsimd.partition_id()
            coff = pid * rows
            n40_rot = const.tile([128, jt_n], dt.float32)
            nc.gpsimd.dma_start(
                out=n40_rot[:].rearrange("q (c t) -> q c t", t=nrt),
                in_=n40_dbl[bass.ds(coff, npan * rows)].rearrange(
                    "(c p t) -> p c t", p=128, t=nrt
                ),
            )

            # rhs partial: sum exp(norms/38) over own rows
            rs = stat.tile([128, 1], dt.float32)
            trash_n = stat.tile([128, nrt], dt.float32)
            nc.scalar.activation(
                trash_n[:], ns[:], AF.Exp, scale=1.0 / (4.0 * BETA - 2.0),
                accum_out=rs[:],
            )

            # PE-bias moving rows: bf16 two-term split of r = n/2 - 256
            rf = stat.tile([128, nrt], dt.float32)
            nc.scalar.activation(
                rf[:], ns[:], AF.Identity, scale=0.5, bias=cm256[:]
            )
            nc.scalar.activation(comb[:, 16:32], rf[:], AF.Copy)
            r1f = stat.tile([128, nrt], dt.float32)
            nc.scalar.activation(r1f[:], comb[:, 16:32], AF.Copy)
            r2f = stat.tile([128, nrt], dt.float32)
            nc.vector.tensor_tensor(
                out=r2f[:], in0=rf[:], in1=r1f[:], op=ALU.subtract
            )
            nc.scalar.activation(comb[:, 32:48], r2f[:], AF.Copy)

            r12 = const.tile([2, rows], red_dt)
            ones2 = const.tile([2, 128], red_dt)
            nc.vector.memset(ones2[:], 1.0)

            # ---------------- main loop ----------------
            def chunks(m0, m1):
                out = []
                while m0 < m1:
                    m2 = min((m0 // 512 + 1) * 512, m1)
                    out.append((m0, m2))
                    m0 = m2
                return out

            acc = accp.tile([128, n_acc], dt.float32)
            w_bc = const.tile([128, rows], red_dt)
            n40_rot2 = const.tile([128, jt_n], dt.float32)
            n40_rot2c = const.tile([128, jt_n], dt.float32)
            n40_w4 = const.tile([128, nrt], dt.float32)
            corr48 = stat.tile([128, 48], dt.float32)
            corr16 = stat.tile([128, nrt], dt.float32)
            deferred = []

            def emit_tail(col, et, mlo, mhi=W):
                tt_o = ttp.tile([128, W], red_dt, tag="tt")
                nc.vector.scalar_tensor_tensor(
                    out=tt_o[:, mlo:mhi], in0=et[:, mlo:mhi], scalar=1.0,
                    in1=w_bc[:, mlo:mhi], op0=ALU.mult, op1=ALU.mult,
                    accum_out=acc[:, col : col + 1],
                )

            def emit_corr_tables():
                # AllGather-dependent bias tables; emitted mid-stream,
                # long after the collective has completed
                nc.scalar.activation(
                    n40_rot2[:], n40_rot[:], AF.Identity, bias=ln2c[:]
                )
                nc.scalar.activation(
                    n40_rot2c[:], n40_rot2[:], AF.Identity, bias=c128b[:]
                )
                n40_w4x = const.tile([128, 24], dt.float32)
                nc.sync.dma_start(out=n40_w4x[:, 0:16], in_=n40_rot2[:, 64:80])
                nc.sync.dma_start(out=n40_w4x[:, 16:24], in_=n40_rot2[:, 64:72])
                tmp = nc.gpsimd.alloc_register("sw4reg")
                nc.gpsimd.reg_load(tmp, sw4.ap()[0:1, 0:1])
                troff = nc.gpsimd.snap(tmp, donate=True, min_val=0, max_val=8)
                nc.gpsimd.dma_start(
                    out=n40_w4[:], in_=n40_w4x[:, bass.ds(troff, 16)]
                )
                argt = stat.tile([128, 48], dt.float32)
                nc.scalar.activation(argt[:], n40_rot2[:, 16:64], AF.Copy)
                for jt in range(nrt, 4 * nrt):
                    if PE_NTH > 0 and (jt - nrt) % PE_NTH == 2:
                        nc.scalar.activation(
                            argt[:, jt - nrt : jt - nrt + 1],
                            n40_rot2c[:, jt : jt + 1],
                            AF.Copy,
                        )
                nc.scalar.activation(corr48[:], argt[:], AF.Exp)
                nc.scalar.activation(corr16[:], n40_w4[:], AF.Exp)

            wts = None
            for i, (kind, idx) in enumerate(sched):
                if i % 8 == 0 and i // 8 < len(gseq):
                    g = gseq[i // 8]
                    wts = wpref.pop(g)
                    if i // 8 + 4 < len(gseq):
                        nxt = gseq[i // 8 + 4]
                        if nxt not in wpref:
                            wpref[nxt] = load_wts_group(*wgroups[nxt])

                if kind == "w0":
                    t = idx
                    mlo = 128 * t
                    mm = [(m0, m1, (t % 8) * 128) for m0, m1 in chunks(mlo, W)]
                    acts = [(mlo, mlo + 128, ns40[:, t : t + 1])]
                    if t < nrt - 1:
                        acts.append((mlo + 128, W, ns40_2[:, t : t + 1]))
                elif kind == "w123":
                    jt = idx
                    mlo = 0
                    jcol = ((jt - nrt) % 8) * 128
                    mm = [(m0, m1, jcol) for m0, m1 in chunks(0, W)]
                    acts = [(0, W, 0.0)]
                else:
                    s = idx
                    mlo = 0
                    mm = [(m0, m1, s * 128) for m0, m1 in chunks(0, 1024)] + [
                        (m0, m1, (s + 8) * 128) for m0, m1 in chunks(1024, W)
                    ]
                    acts = [(0, 1024, 0.0), (1024, W, 0.0)]

                is_pe = (
                    PE_NTH > 0
                    and kind == "w123"
                    and (idx - nrt) % PE_NTH == 2
                )
                ps = mainps.tile([128, W], dt.float32, tag="ps")
                for m0, m1, jcol in mm:
                    h = m0 // 1024
                    for kp in range(kc // 2):
                        nc.tensor.matmul(
                            ps[:, m0:m1],
                            wts[kp][:, :, jcol : jcol + 128],
                            mts_half[(kp, h)][:, :, m0 - 1024 * h : m1 - 1024 * h],
                            start=(kp == 0),
                            stop=(kp == kc // 2 - 1) and not is_pe,
                            perf_mode=mybir.MatmulPerfMode.DoubleRow,
                        )
                    if is_pe:
                        nc.tensor.matmul(
                            ps[:, m0:m1],
                            ones2[:],
                            r12[:, m0:m1],
                            start=False,
                            stop=True,
                        )
                et = etp.tile([128, W], red_dt, tag="et")
                if is_pe:
                    nc.scalar.activation(
                        et[:],
                        ps[:],
                        AF.Exp,
                        scale=1.0 / (2.0 * BETA),
                        accum_out=acc[:, i : i + 1],
                    )
                    if i == CORR_AT:
                        emit_corr_tables()
                    continue
                for m0, m1, bias_ap in acts:
                    nc.scalar.activation(
                        et[:, m0:m1],
                        ps[:, m0:m1],
                        AF.Exp,
                        bias=bias_ap,
                        scale=1.0 / (2.0 * BETA),
                    )

                if i < nrt:
                    # defer w0 STT tails until w_bc exists
                    deferred.append((i, et, mlo))
                    if i == nrt - 1:
                        tps = mainps.tile([128, 2048], red_dt, tag="ps")
                        nc.tensor.transpose(
                            tps[:, 0:128], comb[:], identT[:]
                        )
                        nc.scalar.activation(
                            combT[:], tps[:, 0:128], AF.Copy
                        )
                        nc.scalar.dma_start(
                            out=w_row[0:1, :], in_=combT[0:16, :]
                        )
                        nc.scalar.dma_start(
                            out=r12[0:1, :], in_=combT[16:32, :]
                        )
                        nc.scalar.dma_start(
                            out=r12[1:2, :], in_=combT[32:48, :]
                        )
                        wps = mainps.tile([128, W], dt.float32, tag="ps")
                        for b in range(W // 512):
                            nc.tensor.matmul(
                                wps[:, b * 512 : (b + 1) * 512],
                                ones_r[:],
                                w_row[0:1, b * 512 : (b + 1) * 512],
                                start=True,
                                stop=True,
                            )
                        nc.scalar.activation(w_bc[:], wps[:], AF.Copy)
                        for d_i, d_et, d_mlo in deferred:
                            emit_tail(d_i, d_et, d_mlo)
                elif kind == "w4":
                    emit_tail(64 + idx, et, 0, 1024)
                    emit_tail(72 + idx, et, 1024, W)
                else:
                    emit_tail(i, et, mlo)
                if i == CORR_AT:
                    emit_corr_tables()

            # ---------------- final column correction + reduction -------
            cacc = stat.tile([128, 64], dt.float32)
            nc.vector.tensor_tensor(
                out=cacc[:, 0:48], in0=acc[:, 16:64], in1=corr48[:],
                op=ALU.mult,
            )
            nc.vector.tensor_tensor(
                out=cacc[:, 48:64], in0=acc[:, 64:80], in1=corr16[:],
                op=ALU.mult,
            )
            af0 = stat.tile([128, 1], dt.float32)
            nc.vector.tensor_reduce(
                out=af0[:], in_=acc[:, 0:16], op=ALU.add,
                axis=mybir.AxisListType.X,
            )
            af1 = stat.tile([128, 1], dt.float32)
            nc.vector.tensor_reduce(
                out=af1[:], in_=cacc[:], op=ALU.add, axis=mybir.AxisListType.X
            )
            af = stat.tile([128, 1], dt.float32)
            nc.vector.tensor_tensor(
                out=af[:], in0=af0[:], in1=af1[:], op=ALU.add
            )
            nc.sync.dma_start(out=po_lhs, in_=af[:])
            nc.sync.dma_start(out=po_rhs, in_=rs[:])

    nc.compile()
    return nc


_NC_CACHE = None


def _get_nc():
    global _NC_CACHE
    if _NC_CACHE is None:
        _NC_CACHE = build_program()
    return _NC_CACHE


def _run(x: np.ndarray, **spmd_kwargs):
    assert x.shape == (N, D)
    x = np.asarray(x, dtype=np.float32)
    xT = np.ascontiguousarray(x.T)
    wT_bf = xT.astype(ml_dtypes.float8_e4m3)

    in_maps = []
    for c in range(NCORES):
        sl = slice(c * ROWS, (c + 1) * ROWS)
        stg = np.roll(wT_bf, -c * ROWS, axis=1)[:, : (NCORES // 2 + 1) * ROWS]
        if c >= NCORES // 2:
            # swap the w4 panel halves so cores c and c+4 jointly cover all
            # four quadrants of their shared block pair
            w4 = stg[:, 4 * ROWS :].copy()
            stg = np.concatenate(
                [stg[:, : 4 * ROWS], w4[:, ROWS // 2 :], w4[:, : ROWS // 2]],
                axis=1,
            )
        in_maps.append(
            {
                "wT": np.ascontiguousarray(stg),
                "xo": np.ascontiguousarray(
                    x[sl].astype(ml_dtypes.bfloat16)
                ),
                "sw4": np.array(
                    [[8 if c >= NCORES // 2 else 0]], dtype=np.uint32
                ),
                "ident": np.eye(128, dtype=ml_dtypes.bfloat16),
            }
        )

    nc = _get_nc()
    res = run_bass_kernel_spmd(nc, in_maps, core_ids=list(range(NCORES)), **spmd_kwargs)

    lhs_tot = np.float32(0.0)
    rhs_tot = np.float32(0.0)
    for c in range(NCORES):
        lanes = np.asarray(res.results[c]["po"], dtype=np.float32).reshape(-1)
        lhs_tot = np.float32(lhs_tot + lanes[0:128].sum(dtype=np.float32))
        rhs_tot = np.float32(rhs_tot + lanes[128:256].sum(dtype=np.float32))

    # mirror the reference's f32 arithmetic (both coefficients underflow to 0)
    with np.errstate(under="ignore"):
        coef_l = np.float32(1.0 / BETA ** (D / 2))
        coef_r = np.float32(2.0 / (BETA - 0.5) ** (D / 2))
    out = np.float32(coef_l * lhs_tot / np.float32(N) - coef_r * rhs_tot)
    return out, res, (lhs_tot, rhs_tot)


def kernel(x: np.ndarray) -> np.ndarray:
    out, _, _ = _run(x)
    return out


def kernel_traced(x: np.ndarray, trace_cores=None):
    out, res, sums = _run(
        x,
        trace=True,
        trace_cores=trace_cores if trace_cores is not None else [0],
    )
    return out, res, sums
